# revision 11
# baseline (speedup 1.0000x reference)
"""Trainium2 Bass kernel for the differential-LSTM layer (DLSTM).

Problem shapes (hardcoded):
  x  [128, 512, 64] f32, W [64, 2048], U [512, 2048], b [2048], Wd [64, 512]
Returns (hidden_seq [128,512,512], (h_T [128,512], c_T [128,512])).

Sharding: data-parallel over batch across 8 NeuronCores (16 rows per core),
params replicated; the T=512 scan is local per core.

Per-core design ("transposed state" form):
  The recurrent state h is kept TRANSPOSED and packed: hT [128, 4*16]
  (H-chunk major, batch minor), so it feeds the gate matmuls directly as
  the moving operand and all elementwise work runs on [128, 64] tiles.
  Gate pre-activations come out as gates.T via 16 m-block matmuls of N=16
  with U (bf16, cast once) as the stationary operand, accumulating over
  4 K-chunks of U plus an x-side K-chunk ([x_t; 1] @ [W(+Wd on o); b])
  and, for the o-gate, a -Wd @ x_{t-1} correction chunk.  x.T lives in
  SBUF for the whole scan (built once via PE transposes), so the scan
  needs no DRAM reads at all.  h_t is PE-transposed back to batch-major
  each step for a contiguous DMA into hs.
"""

import sys

sys.path.insert(0, "/opt/trn_rl_repo")

import numpy as np

import concourse.bass as bass
import concourse.tile as tile
from concourse import bacc, mybir
from concourse.bass import ds, ts
from concourse.bass_utils import run_bass_kernel_spmd
from concourse.masks import make_identity

N_CORES = 8
B, T, E, H = 128, 512, 64, 512
BL = B // N_CORES          # 16 batch rows per core
G = 4 * H                  # 2048 gate columns
KC = H // 128              # 4 contraction chunks of U
MB = G // 128              # 16 output m-blocks
F32 = mybir.dt.float32
BF16 = mybir.dt.bfloat16
AF = mybir.ActivationFunctionType

UNROLL = 8


def build_nc(nt: int = T, mm_dtype=BF16, unroll: int = UNROLL):
    """Build the per-core Bass program for an nt-step scan."""
    nc = bacc.Bacc("TRN2", target_bir_lowering=False, debug=False,
                   num_devices=N_CORES)

    x_d = nc.dram_tensor("x", [BL, T, E], F32, kind="ExternalInput")
    W_d = nc.dram_tensor("W", [E, G], F32, kind="ExternalInput")
    U_d = nc.dram_tensor("U", [H, G], F32, kind="ExternalInput")
    b_d = nc.dram_tensor("b", [G], F32, kind="ExternalInput")
    Wd_d = nc.dram_tensor("Wd", [E, H], F32, kind="ExternalInput")
    hs_d = nc.dram_tensor("hs", [BL, nt, H], F32, kind="ExternalOutput")
    cT_d = nc.dram_tensor("cT", [BL, H], F32, kind="ExternalOutput")

    NB = BL * T // 128     # 64 blocks of 128 (b,t) columns

    with tile.TileContext(nc) as tc:
        persist_cm = tc.tile_pool(name="persist", bufs=1)
        persist = persist_cm.__enter__()

        ident = persist.tile([128, 128], F32)
        make_identity(nc, ident[:])

        # persistent operands (mm dtype)
        xTb = persist.tile([E + 1, BL * T], mm_dtype)   # x.T, (b t) cols + ones row
        xsb = persist.tile([E, BL * T], mm_dtype)       # x_{t-1}.T
        Wcatb = persist.tile([E + 1, G], mm_dtype)      # [W (+Wd on o-cols); b]
        nWdb = persist.tile([E, H], mm_dtype)           # -Wd
        Ub = persist.tile([128, KC * G], mm_dtype)      # U, K-chunk major
        # states
        hTmm = persist.tile([128, KC * BL], mm_dtype)
        cst = persist.tile([128, KC * BL], F32)

        # ---------------- Phase A: x.T via PE transposes ----------------
        with (
            tc.tile_pool(name="xprep", bufs=1) as xp,
            tc.tile_pool(name="tpsum", bufs=4, space="PSUM") as tp,
        ):
            xn = xp.tile([128, NB, E], F32)
            nc.sync.dma_start(
                out=xn[:],
                in_=x_d.ap().rearrange("b t e -> (b t) e")
                           .rearrange("(n p) e -> p n e", p=128),
            )
            xT32 = xp.tile([E + 1, BL * T], F32)
            for n in range(NB):
                pxt = tp.tile([E, 128], F32, tag="pxt")
                nc.tensor.transpose(pxt[:], xn[:, n, :], ident[:])
                nc.vector.tensor_copy(out=xT32[0:E, ts(n, 128)], in_=pxt[:])
            nc.vector.memset(xT32[E:E + 1, :], 1.0)

            xs32 = xp.tile([E, BL * T], F32)
            xs_r = xs32[:].rearrange("e (b t) -> e b t", b=BL)
            xT_r = xT32[0:E, :].rearrange("e (b t) -> e b t", b=BL)
            nc.vector.tensor_copy(out=xs_r[:, :, 1:T], in_=xT_r[:, :, 0:T - 1])
            nc.vector.tensor_copy(out=xs_r[:, :, 0:1], in_=xT_r[:, :, 0:1])

            nc.vector.tensor_copy(out=xTb[:], in_=xT32[:])
            nc.vector.tensor_copy(out=xsb[:], in_=xs32[:])

        # ---------------- Phase B: W/b/Wd/U load + cast ----------------
        with tc.tile_pool(name="wprep", bufs=1) as wp:
            Wcat32 = wp.tile([E + 1, G], F32)
            nc.sync.dma_start(out=Wcat32[0:E, :], in_=W_d.ap())
            nc.sync.dma_start(
                out=Wcat32[E:E + 1, :],
                in_=b_d.ap().rearrange("(o g) -> o g", o=1),
            )
            WdS = wp.tile([E, H], F32)
            nc.sync.dma_start(out=WdS[:], in_=Wd_d.ap())
            nc.vector.tensor_add(
                out=Wcat32[0:E, 3 * H:4 * H],
                in0=Wcat32[0:E, 3 * H:4 * H],
                in1=WdS[:],
            )
            nc.vector.tensor_copy(out=Wcatb[:], in_=Wcat32[:])
            nc.scalar.mul(WdS[:], WdS[:], -1.0)
            nc.vector.tensor_copy(out=nWdb[:], in_=WdS[:])

            U32 = wp.tile([128, KC * G], F32)
            nc.sync.dma_start(
                out=U32[:].rearrange("p (k g) -> p k g", k=KC),
                in_=U_d.ap().rearrange("(k p) g -> p k g", p=128),
            )
            nc.vector.tensor_copy(out=Ub[:], in_=U32[:])

        nc.vector.memset(hTmm[:], 0.0)
        nc.vector.memset(cst[:], 0.0)

        # ---------------- Phase C: the scan ----------------
        xT_v = xTb[:].rearrange("e (b t) -> e t b", b=BL)
        xs_v = xsb[:].rearrange("e (b t) -> e t b", b=BL)

        with (
            tc.tile_pool(name="sc", bufs=3) as sc,
            tc.tile_pool(name="spsum", bufs=1, space="PSUM") as sps,
            tc.tile_pool(name="hpsum", bufs=2, space="PSUM") as hps,
        ):
            def step(t):
                xc_t = xT_v[:, ds(t, 1), :].squeeze(1)   # [65, 16]
                xp_t = xs_v[:, ds(t, 1), :].squeeze(1)   # [64, 16]

                acts = []
                for g in range(4):
                    psg = sps.tile([128, 4 * BL], F32, tag=f"ps{g}")
                    for ml in range(4):
                        mb = g * 4 + ml
                        o = psg[:, ml * BL:(ml + 1) * BL]
                        # x-side chunk first: doesn't depend on h_{t-1}
                        nc.tensor.matmul(
                            o, lhsT=Wcatb[:, mb * 128:(mb + 1) * 128],
                            rhs=xc_t, start=True, stop=False)
                        if g == 3:
                            nc.tensor.matmul(
                                o, lhsT=nWdb[:, ml * 128:(ml + 1) * 128],
                                rhs=xp_t, start=False, stop=False)
                        for kc in range(KC):
                            nc.tensor.matmul(
                                o,
                                lhsT=Ub[:, kc * G + mb * 128:
                                        kc * G + (mb + 1) * 128],
                                rhs=hTmm[:, kc * BL:(kc + 1) * BL],
                                start=False, stop=(kc == KC - 1))
                    act = sc.tile([128, 4 * BL], F32, tag=f"act{g}")
                    nc.scalar.activation(
                        act[:], psg[:], AF.Tanh if g == 2 else AF.Sigmoid)
                    acts.append(act)

                a_i, a_f, a_g, a_o = acts
                ig = sc.tile([128, KC * BL], F32, tag="ig")
                nc.vector.tensor_mul(out=ig[:], in0=a_i[:], in1=a_g[:])
                nc.vector.tensor_mul(out=cst[:], in0=a_f[:], in1=cst[:])
                nc.vector.tensor_add(out=cst[:], in0=cst[:], in1=ig[:])
                tch = sc.tile([128, KC * BL], F32, tag="tch")
                nc.scalar.activation(tch[:], cst[:], AF.Tanh)
                hst = sc.tile([128, KC * BL], F32, tag="hst")
                nc.vector.tensor_mul(out=hst[:], in0=a_o[:], in1=tch[:])
                nc.vector.tensor_copy(out=hTmm[:], in_=hst[:])

                # transpose h back to batch-major and store
                pht = hps.tile([BL, H], F32, tag="pht")
                for kc in range(KC):
                    nc.tensor.transpose(
                        pht[:, kc * 128:(kc + 1) * 128],
                        hst[:, kc * BL:(kc + 1) * BL], ident[:])
                hrow = sc.tile([BL, H], F32, tag="hrow")
                nc.vector.tensor_copy(out=hrow[:], in_=pht[:])
                nc.sync.dma_start(
                    out=hs_d.ap()[:, ds(t, 1), :].squeeze(1), in_=hrow[:])

            if nt > unroll:
                assert nt % unroll == 0
                with tc.For_i(0, nt, unroll,
                              hint_engines=(mybir.EngineType.PE,)) as t0:
                    for dt in range(unroll):
                        step(t0 + dt)
            else:
                for t in range(nt):
                    step(t)

            # final c, transposed to batch-major
            pct = hps.tile([BL, H], F32, tag="pht")
            for kc in range(KC):
                nc.tensor.transpose(
                    pct[:, kc * 128:(kc + 1) * 128],
                    cst[:, kc * BL:(kc + 1) * BL], ident[:])
            crow = sc.tile([BL, H], F32, tag="hrow")
            nc.vector.tensor_copy(out=crow[:], in_=pct[:])
            nc.sync.dma_start(out=cT_d.ap(), in_=crow[:])

        persist_cm.__exit__(None, None, None)

    nc.compile()
    return nc


_NC_CACHE = {}


def _get_nc(nt=T, mm_dtype=BF16, unroll=UNROLL):
    key = (nt, str(mm_dtype), unroll)
    if key not in _NC_CACHE:
        _NC_CACHE[key] = build_nc(nt, mm_dtype, unroll)
    return _NC_CACHE[key]


def _in_maps(inputs):
    x = np.ascontiguousarray(np.asarray(inputs["x"], dtype=np.float32))
    W = np.ascontiguousarray(np.asarray(inputs["W"], dtype=np.float32))
    U = np.ascontiguousarray(np.asarray(inputs["U"], dtype=np.float32))
    b = np.ascontiguousarray(np.asarray(inputs["b"], dtype=np.float32))
    Wd = np.ascontiguousarray(np.asarray(inputs["Wd"], dtype=np.float32))
    return [
        {"x": x[i * BL:(i + 1) * BL], "W": W, "U": U, "b": b, "Wd": Wd}
        for i in range(N_CORES)
    ]


def kernel(**inputs):
    nc = _get_nc()
    res = run_bass_kernel_spmd(nc, _in_maps(inputs),
                               core_ids=list(range(N_CORES)))
    hs = np.concatenate([res.results[i]["hs"] for i in range(N_CORES)], axis=0)
    cT = np.concatenate([res.results[i]["cT"] for i in range(N_CORES)], axis=0)
    hT = np.ascontiguousarray(hs[:, -1, :])
    return hs, (hT, cT)
